# revision 43
# baseline (speedup 1.0000x reference)
"""Trainium2 Bass kernel for nn_Attention_b (tanh-attention with masked_scatter).

Data-parallel over batch: each of 8 NeuronCores owns 4 batches. Per core:
  sweep A  z = W1 @ h_i + (W2 @ h_t + b)   (fp16 GEMM, [A, rows])
           m = tanh(z); y = u . m          (raw scores, [rows])
           AllGather of score pairs across the 8 cores (pipelined)
           then per pair: masked_scatter selection (0/1 matrix against
           gathered scores), chunk-local max + exp + running sums
  sweep B  chunk weights wn = exp(m_i - M)/L; scatter e*wn onto
           s-partitions via rank-1 PE matmuls (transpose for free);
           s[b, :] = sum_s e~ * h_iT streams on the PE (e columns
           stationary, per-b output rows at PSUM partitions 32*b).
h_i is sent twice (fp16): once h-major for the GEMM, once s-major for
the weighted sum, so nothing transposes the big tensor on chip.
"""
import sys

for _p in ("/opt/trn_rl_repo",):
    if _p not in sys.path:
        sys.path.insert(0, _p)

import numpy as np

import concourse.bacc as bacc
import concourse.tile as tile
from concourse import mybir
from concourse.bass_utils import run_bass_kernel_spmd

NCORES = 8
B, S, H, A = 32, 2048, 1024, 256
BL = B // NCORES          # local batches per core
NEG = np.float32(-1e20)

f32 = mybir.dt.float32
f32r = mybir.dt.float32r
f16 = mybir.dt.float16

def build_kernel(S=S, H=H, A=A, C=256, hi_bufs=3, hit_bufs=5):
    KT = H // 128             # contraction tiles
    AT = A // 128             # score tiles
    NCH = S // C              # chunks (phase-1/2 granularity)
    NPR = NCH // 2            # AllGather pairs
    NSB = S // 128            # seq 128-blocks (phase-3 granularity)
    SBC = C // 128            # seq blocks per chunk
    assert H % 128 == 0 and A % 128 == 0 and S % (2 * C) == 0

    nc = bacc.Bacc("TRN2", target_bir_lowering=False, debug=False,
                   num_devices=NCORES)

    hi5 = nc.declare_dram_parameter("hi5", [128, KT * BL * S], f16,
                                    isOutput=False)
    hit = nc.declare_dram_parameter("hit", [128, NSB * BL * H], f16,
                                    isOutput=False)
    w1t = nc.declare_dram_parameter("w1t", [H, A], f16, isOutput=False)
    cb2 = nc.declare_dram_parameter("cb2", [128, AT, BL], f32, isOutput=False)
    u2 = nc.declare_dram_parameter("u2", [128, AT], f16, isOutput=False)
    sel = nc.declare_dram_parameter("sel", [B + 1, BL, S], f16,
                                    isOutput=False)
    out = nc.declare_dram_parameter("out", [BL, H], f32, isOutput=True)

    with tile.TileContext(nc) as tc:
        with (
            tc.tile_pool(name="consts", bufs=1) as cp,
            tc.tile_pool(name="hi", bufs=hi_bufs) as hip,
            tc.tile_pool(name="hit", bufs=hit_bufs) as htp,
            tc.tile_pool(name="m", bufs=2) as mp,
            tc.tile_pool(name="small", bufs=3) as sp,
            tc.tile_pool(name="ps", bufs=4, space="PSUM") as pp,
            tc.tile_pool(name="dram", bufs=12, space="DRAM") as dp,
        ):
            # ---- preload replicated constants
            w1_sb = cp.tile([128, KT, A], f16)
            nc.scalar.dma_start(
                out=w1_sb, in_=w1t.rearrange("(t p) a -> p t a", p=128))
            u_sb = cp.tile([128, AT], f16)
            nc.scalar.dma_start(out=u_sb, in_=u2[:, :])
            cb_sb = cp.tile([128, AT, BL], f32)
            nc.scalar.dma_start(out=cb_sb, in_=cb2[:, :, :])
            ones_sb = cp.tile([B + 1, 1], f16)
            nc.vector.memset(ones_sb, 1.0)
            one1 = cp.tile([1, 1], f32)
            nc.vector.memset(one1, 1.0)

            # ---- per-chunk softmax stats + resident exp values
            mall = cp.tile([1, BL, NCH], f32)
            lall = cp.tile([1, BL, NCH], f32)
            e_all = cp.tile([1, BL, S], f16)

            def phase1(i):
                off = C * i
                hi_sb = hip.tile([128, KT, BL, C], f16, tag="hi")
                nc.sync.dma_start(
                    out=hi_sb.rearrange("p t b s -> p (t b s)"),
                    in_=hi5[:, KT * BL * off : KT * BL * (off + C)])
                sel_c = sp.tile([B + 1, BL, C], f16, tag="selc", bufs=NCH)
                nc.scalar.dma_start(out=sel_c, in_=sel[:, :, off : off + C])
                m_r = mp.tile([128, AT, BL, C], f16, tag="m")
                for at in range(AT):
                    z_ps = pp.tile([128, BL, C], f32, tag="zy")
                    for kt in range(KT):
                        for r in range(BL // 2):
                            nc.tensor.matmul(
                                z_ps[:, 2 * r : 2 * r + 2, :],
                                w1_sb[:, kt, at * 128 : (at + 1) * 128],
                                hi_sb[:, kt, 2 * r : 2 * r + 2, :],
                                start=(kt == 0), stop=(kt == KT - 1),
                            )
                    for b in range(BL):
                        nc.scalar.activation(
                            out=m_r[:, at, b, :], in_=z_ps[:, b, :],
                            func=mybir.ActivationFunctionType.Tanh,
                            bias=cb_sb[:, at, b : b + 1], scale=1.0,
                        )
                return dict(m_r=m_r, sel_c=sel_c, i=i)

            def phase1y(c):
                m_r = c.pop("m_r")
                y_ps = pp.tile([1, BL, C], f32, tag="zy")
                for at in range(AT):
                    for r in range(BL // 2):
                        nc.tensor.matmul(
                            y_ps[:, 2 * r : 2 * r + 2, :],
                            u_sb[:, at : at + 1],
                            m_r[:, at, 2 * r : 2 * r + 2, :],
                            start=(at == 0), stop=(at == AT - 1),
                        )
                c["y_ps"] = y_ps
                return c

            def aggather(cs):
                """Copy the group's chunk scores out and AllGather them."""
                g = len(cs)
                y_sb = sp.tile([1, BL, g, C], f16, tag="ysb", bufs=1)
                for k, c in enumerate(cs):
                    nc.scalar.activation(
                        out=y_sb[:, :, k, :], in_=c["y_ps"],
                        func=mybir.ActivationFunctionType.Copy)
                ag_in = dp.tile([g * BL * C], f16, tag="agin")
                nc.scalar.dma_start(
                    out=ag_in.rearrange("(o n) -> o n", o=1),
                    in_=y_sb.rearrange("p b c s -> p (b c s)"))
                ag_out = dp.tile([g * B * C], f16, tag="agout",
                                 addr_space="Shared")
                nc.gpsimd.collective_compute(
                    "AllGather", mybir.AluOpType.bypass,
                    ins=[ag_in[:]], outs=[ag_out[:]],
                    replica_groups=[list(range(NCORES))],
                )
                return dict(ag_out=ag_out, cs=cs)

            def reload(ag):
                # gathered rows: [(core, b), (chunk-of-group, s)]
                g = len(ag["cs"])
                y32 = sp.tile([B + 1, g, C], f16, tag="y32", bufs=4)
                nc.gpsimd.memset(y32[B : B + 1, :, :], 1.0)
                nc.gpsimd.dma_start(
                    out=y32[:B].rearrange("j c s -> j (c s)"),
                    in_=ag["ag_out"].rearrange("(j n) -> j n", n=g * C))
                return y32

            def phase2(c, y32):
                i, sel_c = c["i"], c["sel_c"]
                # masked_scatter selection: one-hot rows (plus a -1e20 mask
                # row) dotted with [y; 1]
                bt_ps = pp.tile([1, BL, C], f32, tag="zy")
                selY = sp.tile([B + 1, BL, C], f16, tag="selY", bufs=2)
                nc.vector.tensor_mul(
                    selY, sel_c,
                    y32.rearrange("j (o s) -> j o s", o=1)
                       .broadcast_to([B + 1, BL, C]))
                for hf in range(2):
                    nc.tensor.matmul(
                        bt_ps[:, 2 * hf : 2 * hf + 2, :], ones_sb,
                        selY[:, 2 * hf : 2 * hf + 2, :],
                        start=True, stop=True)

                # chunk-local max -> no cross-chunk recurrence
                cmax = sp.tile([1, BL], f32, tag="cmax")
                nc.vector.tensor_reduce(
                    out=cmax.rearrange("p (b o) -> p b o", o=1), in_=bt_ps,
                    axis=mybir.AxisListType.X, op=mybir.AluOpType.max)
                nc.vector.tensor_copy(mall[:, :, i], cmax)
                nmnew = sp.tile([1, BL], f32, tag="nmnew")
                nc.vector.tensor_scalar_mul(nmnew, cmax, -1.0)
                for b in range(BL):
                    nc.scalar.activation(
                        out=e_all[:, b, C * i : C * (i + 1)],
                        in_=bt_ps[:, b, :],
                        func=mybir.ActivationFunctionType.Exp,
                        bias=nmnew[:, b : b + 1], scale=1.0,
                        accum_out=lall[:, b, i : i + 1])

            # ---- sweep A: scores + pipelined AllGathers (no phase2 yet, so
            # the Act queue never blocks on a collective; each chunk's y
            # matmuls issue after the next chunk's z so the PE never stalls
            # on the tanh)
            GS = [2, 2, 2, 2]
            ends = [sum(GS[: k + 1]) for k in range(len(GS))]
            pairs = []
            done, prev = [], None
            for i in range(NCH):
                cur = phase1(i)
                if prev is not None:
                    done.append(phase1y(prev))
                    if len(done) in ends:
                        k = ends.index(len(done))
                        pairs.append(aggather(done[len(done) - GS[k]:]))
                prev = cur
            done.append(phase1y(prev))
            k = len(GS) - 1
            pairs.append(aggather(done[NCH - GS[k]:]))

            # ---- stream the s-major batch-interleaved copy of h_i
            hit_tiles = []
            for j in range(NCH):
                ht = htp.tile([128, SBC, BL, H], f16, tag="hit")
                nc.sync.dma_start(
                    out=ht.rearrange("p c b h -> p (c b h)"),
                    in_=hit[:, SBC * BL * H * j : SBC * BL * H * (j + 1)])
                hit_tiles.append(ht)

            # ---- per-pair: phase 2 + pair-local eT + phase-3 partial,
            # flash-style running-max rescale into an SBUF accumulator
            acc = cp.tile([128, H], f32)
            Mr = cp.tile([1, BL], f32)
            gbase = 0
            for p, ag in enumerate(pairs):
                g = len(ag["cs"])
                y32 = reload(ag)
                for k, c in enumerate(ag["cs"]):
                    phase2(c, y32[:, k, :])
                # group max and running max
                pmax = sp.tile([1, BL], f32, tag="pmax")
                nc.vector.tensor_reduce(
                    out=pmax.rearrange("p (b o) -> p b o", o=1),
                    in_=mall[:, :, gbase : gbase + g],
                    axis=mybir.AxisListType.X, op=mybir.AluOpType.max)
                if p == 0:
                    nc.vector.tensor_copy(Mr, pmax)
                else:
                    Mrn = sp.tile([1, BL], f32, tag="mrn")
                    nc.vector.tensor_max(Mrn, Mr, pmax)
                    dr = sp.tile([1, BL], f32, tag="dr")
                    nc.vector.tensor_sub(dr, Mr, Mrn)
                    rold = sp.tile([1, BL], f32, tag="rold")
                    nc.scalar.activation(
                        out=rold, in_=dr,
                        func=mybir.ActivationFunctionType.Exp, scale=1.0)
                    nc.vector.tensor_copy(Mr, Mrn)
                    # broadcast rold to one value per partition row (32b+j)
                    r128 = sp.tile([1, 128], f32, tag="r128")
                    nc.vector.tensor_copy(
                        r128.rearrange("p (b j) -> p b j", j=32),
                        dr.rearrange("p (b o) -> p b o", o=1)
                          .broadcast_to([1, BL, 32]))
                    nc.scalar.activation(
                        out=r128, in_=r128,
                        func=mybir.ActivationFunctionType.Exp, scale=1.0)
                    scal_ps = pp.tile([128, 1], f32, tag="zy")
                    nc.tensor.matmul(scal_ps, r128, one1,
                                     start=True, stop=True)
                    scal_sb = sp.tile([128, 1], f32, tag="scal")
                    nc.vector.tensor_copy(scal_sb, scal_ps)
                if p == len(pairs) - 1:
                    # 1/L depends only on the (now final) stats; issue it
                    # here so the DVE/Act work hides under pair-3's PE work
                    dL = sp.tile([1, BL, NCH], f32, tag="dL")
                    nc.vector.tensor_sub(
                        dL, mall,
                        Mr.rearrange("p (b o) -> p b o", o=1)
                          .broadcast_to([1, BL, NCH]))
                    nc.scalar.activation(
                        out=dL, in_=dL,
                        func=mybir.ActivationFunctionType.Exp, scale=1.0)
                    wl = sp.tile([1, BL, NCH], f32, tag="wl")
                    nc.vector.tensor_mul(wl, dL, lall)
                    lsum = sp.tile([1, BL], f32, tag="lsum")
                    nc.vector.tensor_reduce(
                        out=lsum.rearrange("p (b o) -> p b o", o=1), in_=wl,
                        axis=mybir.AxisListType.X, op=mybir.AluOpType.add)
                    il = sp.tile([1, BL], f32, tag="il")
                    nc.vector.reciprocal(il, lsum)
                    il128 = sp.tile([1, 128], f32, tag="r128b")
                    nc.vector.tensor_copy(
                        il128.rearrange("p (b j) -> p b j", j=32),
                        il.rearrange("p (b o) -> p b o", o=1)
                          .broadcast_to([1, BL, 32]))
                    ils_ps = pp.tile([128, 1], f32, tag="zy")
                    nc.tensor.matmul(ils_ps, il128, one1,
                                     start=True, stop=True)
                    ils_sb = sp.tile([128, 1], f32, tag="scalL")
                    nc.vector.tensor_copy(ils_sb, ils_ps)
                # group chunk weights relative to running max
                wt = sp.tile([1, BL, g], f32, tag="wt")
                nc.vector.tensor_sub(
                    wt, mall[:, :, gbase : gbase + g],
                    Mr.rearrange("p (b o) -> p b o", o=1)
                      .broadcast_to([1, BL, g]))
                nc.scalar.activation(
                    out=wt, in_=wt,
                    func=mybir.ActivationFunctionType.Exp, scale=1.0)
                wt16 = sp.tile([1, BL, g], f16, tag="wt16")
                nc.vector.tensor_copy(wt16, wt)
                # eT[j, sb, b] = e_all[b, sb*128+j] * wt[b, chunk(sb)]
                eT_ps = pp.tile([128, g * SBC, BL], f32, tag="zy")
                for sl in range(g * SBC):
                    sb = SBC * gbase + sl
                    for b in range(BL):
                        nc.tensor.matmul(
                            eT_ps[:, sl, b : b + 1],
                            e_all[:, b, 128 * sb : 128 * (sb + 1)],
                            wt16[:, b, sl // SBC : sl // SBC + 1],
                            start=True, stop=True)
                eT = sp.tile([128, g * SBC, BL], f16, tag="eT", bufs=2)
                nc.scalar.activation(out=eT, in_=eT_ps,
                                     func=mybir.ActivationFunctionType.Copy)
                # phase-3 partial for this group's chunks
                p3p = pp.tile([128, H], f32, tag="zy")
                HH = H // 512
                for jc in range(g):
                    ht = hit_tiles[gbase + jc]
                    for c in range(SBC):
                        sl = SBC * jc + c
                        for b in range(BL):
                            for hh in range(HH):
                                nc.tensor.matmul(
                                    p3p[32 * b : 32 * b + 1,
                                        512 * hh : 512 * (hh + 1)],
                                    eT[:, sl, b : b + 1],
                                    ht[:, c, b, 512 * hh : 512 * (hh + 1)],
                                    start=(sl == 0), stop=(sl == g * SBC - 1),
                                    tile_position=(0, 32 * b),
                                )
                gbase += g
                if p == 0:
                    nc.vector.tensor_copy(acc, p3p)
                else:
                    nc.vector.tensor_scalar_mul(acc, acc, scal_sb)
                    nc.vector.tensor_add(acc, acc, p3p)

            nc.vector.tensor_scalar_mul(acc, acc, ils_sb)
            nc.sync.dma_start(
                out=out[:, :],
                in_=acc.rearrange("(b r) h -> b r h", r=32)[:, 0, :])

    nc.compile()
    _split_pe_waits(nc)
    return nc


def _split_pe_waits(nc):
    """TRN2 PE instructions (S3_LW encoding) take a single sync-wait slot.
    Bacc's legalization misses some Matmults; hoist excess waits onto
    dedicated PE NoOps inserted directly before the offender."""
    for f in nc.m.functions:
        for bb in f.blocks:
            insts = bb.instructions
            i = 0
            while i < len(insts):
                ins = insts[i]
                if type(ins).__name__ in ("InstMatmult", "InstNoOp") and \
                        ins.engine == mybir.EngineType.PE:
                    si = ins.sync_info
                    if si is not None and len(si.on_wait) > 1:
                        extra, keep = si.on_wait[:-1], si.on_wait[-1:]
                        for w in extra:
                            nop = mybir.InstNoOp(
                                name=nc.get_next_instruction_name(),
                                ins=[], outs=[])
                            nop.engine = ins.engine
                            nop.sync_info = mybir.SyncInfo(
                                on_wait=[w], on_update=[])
                            nc.register_instruction(nop)
                            insts.insert(i, nop)
                            i += 1
                        si.on_wait = keep
                i += 1


def prep_inputs(h_i, h_t, mask, W, b, u, S=S, H=H, A=A, C=256):
    """Shard + lay out the full inputs for the 8 cores."""
    h_i = np.asarray(h_i, np.float32)
    h_t = np.asarray(h_t, np.float32)
    mask = np.asarray(mask, bool)
    W = np.asarray(W, np.float32)
    b = np.asarray(b, np.float32)
    u = np.asarray(u, np.float32)

    KT = H // 128
    AT = A // 128
    NSB = S // 128
    w1t = np.ascontiguousarray(W[:, :H].T).astype(np.float16)   # [H, A]
    cb = h_t @ W[:, H:].T + b                                   # [B, A]
    cb2s = np.ascontiguousarray(
        cb.reshape(B, AT, 128).transpose(2, 1, 0))              # [128, AT, B]
    u2 = np.ascontiguousarray(
        u[:, 0].reshape(AT, 128).T).astype(np.float16)          # [128, AT]

    pos = np.clip(np.cumsum(mask.astype(np.int64), axis=0) - 1, 0, None)
    onehot = (np.arange(B)[None, :, None] == pos[:, None, :]) & mask[:, None, :]
    selall = onehot.astype(np.float32)                          # [B, B, S]
    negall = np.where(mask, np.float32(0), np.float32(-60000.0))
    sel33 = np.concatenate([selall, negall[:, None, :]],
                           axis=1).astype(np.float16)  # [B, B+1, S]

    in_maps = []
    for c in range(NCORES):
        bs = slice(c * BL, (c + 1) * BL)
        hcf = h_i[bs].astype(np.float16)                    # [BL, S, H]
        # hi5[p, chunk_i ++ (t, b, s)] = h_i[b, off_i+s, t*128+p]
        h4 = hcf.reshape(BL, S // C, C, KT, 128)
        hi5 = np.ascontiguousarray(
            h4.transpose(4, 1, 3, 0, 2).reshape(128, KT * BL * S))
        # hit[p, (sb, b, h)] = h_i[b, sb*128+p, h]
        h5 = hcf.reshape(BL, NSB, 128, H)
        hitm = np.ascontiguousarray(
            h5.transpose(2, 1, 0, 3).reshape(128, NSB * BL * H))
        in_maps.append({
            "hi5": hi5,
            "hit": hitm,
            "w1t": w1t,
            "cb2": np.ascontiguousarray(cb2s[:, :, bs]),
            "u2": u2,
            "sel": np.ascontiguousarray(sel33[bs].transpose(1, 0, 2)),
        })
    return in_maps


_NC_CACHE = {}


def _get_nc():
    if "nc" not in _NC_CACHE:
        _NC_CACHE["nc"] = build_kernel()
    return _NC_CACHE["nc"]


def kernel(h_i, h_t, mask, W, b, u):
    nc = _get_nc()
    in_maps = prep_inputs(h_i, h_t, mask, W, b, u)
    res = run_bass_kernel_spmd(nc, in_maps, list(range(NCORES)))
    return np.concatenate([res.results[c]["out"] for c in range(NCORES)],
                          axis=0)
